# revision 3
# baseline (speedup 1.0000x reference)
"""Trainium2 Bass kernel for the SNN (LIF) network:

    cur1 = x.reshape(B,-1) @ W1.T + b1          (big fp32 matmul, once)
    200 sequential LIF steps on [B,1000] (layer 1), tiny matmul into 5
    outputs per step (layer 2), second LIF on [B,5].

Distribution over 8 cores:
  Phase A: contraction(K)-sharded exact-fp32 matmul (3 f16 passes:
           xh*wh + xh*wl + xl*wh) -> per-core partial cur1 [256, 1024],
           ReduceScatter(add) -> each core owns a 32-row batch slice.
  Phase B: per-core LIF layer-1 scan over its 32-batch slice, hidden on
           partitions ([128, 8chunks x 32batch] tiles). One custom DVE
           instruction per step writes into a 4-slot group buffer; one
           ACT Sign per 4 steps emits spikes g=sign(mem-1) in f16.
  Phase C: per group of 4 steps, PE contracts W2 chunks (stationary
           [128,5] f16) against g (moving [128, 4*32]) into PSUM [5,128];
           ACT copies out with scale=0.5 and bias b2+0.5*sum(W2) (folds
           spk=(1+g)/2).
  Phase D: layer-2 LIF scan on [5, 32] per step; spk2 = (mem2 > 1) at the
           end. Outputs [5, T*32] gathered on host.
"""
import sys

if "/opt/trn_rl_repo" not in sys.path:
    sys.path.insert(0, "/opt/trn_rl_repo")

import numpy as np
import ml_dtypes

# ---------------------------------------------------------------- constants
BETA = 0.95
T = 200
B = 256
NIN = 32000
NH = 1000
NO = 5

N_CORES = 8
KPAD = 32768           # NIN padded to 256*128
KC = KPAD // N_CORES   # 4096 contraction per core
KTILES = KC // 128     # 32
KB = 4                 # ktiles per weight DMA batch
NKB = KTILES // KB     # 8 weight DMA rounds
HPAD = 1024            # hidden padded
BLOC = B // N_CORES    # 32 batch rows per core after ReduceScatter
NCHUNK = HPAD // 128   # 8 hidden chunks of 128
G = 4                  # steps per sign/matmul group
NGROUP = T // G        # 50
W1SCALE = 256.0        # W1 pre-scale so the fp16 lo-half stays normal
CB = NCHUNK * BLOC     # 256 elements per partition per step

# ---------------------------------------------------------------- custom op
_LIF_NAME = "LIF_STEP_ANT"


def _register_lif_op():
    from concourse.dve_ops import (
        DveOp, OPS, CUSTOM_DVE_SPECS, _SUB_OPCODE_FOR_NAME, _CUSTOM_DVE_ROW_BASE,
    )
    from concourse.dve_spec import Spec, Src0, Src1, C0, One, lower as dve_lower, _has_src1
    from concourse.dve_uop import DveOpSpec

    for op in OPS:
        if op.name == _LIF_NAME:
            return op
    spec = Spec(
        body=Src0 * C0 + Src1 - (Src0 > One),
        reference=lambda in0, in1, s0: in0 * s0 + in1 - (in0 > 1.0).astype(np.float32),
    )
    if _LIF_NAME not in _SUB_OPCODE_FOR_NAME:
        _SUB_OPCODE_FOR_NAME[_LIF_NAME] = _CUSTOM_DVE_ROW_BASE + len(OPS)
    shas = {}
    for ver in ("v3", "v4"):
        s = DveOpSpec(
            name=_LIF_NAME,
            opcode=_SUB_OPCODE_FOR_NAME[_LIF_NAME],
            uops=dve_lower(spec, ver=ver),
            rd1_en=_has_src1(spec),
        )
        shas[ver] = s.sha(ver)
    op = DveOp(_LIF_NAME, spec, subdim=False, uops_sha=shas)
    OPS.append(op)
    CUSTOM_DVE_SPECS[_LIF_NAME] = op.spec
    return op


# ---------------------------------------------------------------- program
_PROGRAMS = {}  # sim -> (nc, lif_op)


def _build_program(sim=False):
    if sim in _PROGRAMS:
        return _PROGRAMS[sim]

    import concourse.bass as bass
    import concourse.tile as tile
    from concourse import bacc, mybir
    from concourse.masks import make_identity

    LIF = _register_lif_op()
    f32 = mybir.dt.float32
    f16 = mybir.dt.float16

    nc = bacc.Bacc("TRN2", target_bir_lowering=False, debug=False,
                   num_devices=1 if sim else N_CORES)

    # inputs (per-core); x/w1 pre-transposed on host so per-partition DMA
    # lines are long and contiguous.
    xth_d = nc.dram_tensor("xth", [128, KTILES * B], f16, kind="ExternalInput").ap()
    xtl_d = nc.dram_tensor("xtl", [128, KTILES * B], f16, kind="ExternalInput").ap()
    w1h_d = nc.dram_tensor("w1h", [128, KTILES, HPAD], f16, kind="ExternalInput").ap()
    w1l_d = nc.dram_tensor("w1l", [128, KTILES, HPAD], f16, kind="ExternalInput").ap()
    b1c_d = nc.dram_tensor("b1c", [128, NCHUNK], f32, kind="ExternalInput").ap()
    w2s_d = nc.dram_tensor("w2s", [128, NCHUNK, NO], f16, kind="ExternalInput").ap()
    b2c_d = nc.dram_tensor("b2c", [NO, 1], f32, kind="ExternalInput").ap()
    # outputs (per-core batch slice), layout [NO, t*32b]
    mem2_d = nc.dram_tensor("mem2rec", [NO, T * BLOC], f32, kind="ExternalOutput").ap()
    spk2_d = nc.dram_tensor("spk2rec", [NO, T * BLOC], f32, kind="ExternalOutput").ap()

    with tile.TileContext(nc) as tc:
        with (
            tc.tile_pool(name="win", bufs=2) as wpool,
            tc.tile_pool(name="psA", bufs=1, space="PSUM") as psA,
            tc.tile_pool(name="stage", bufs=1) as stage,
            tc.tile_pool(name="dram", bufs=1, space="DRAM") as dram,
            tc.tile_pool(name="g4", bufs=3) as gpool,
            tc.tile_pool(name="psC", bufs=2, space="PSUM") as psC,
            tc.tile_pool(name="psT", bufs=2, space="PSUM") as psT,
        ):
            # ------------- small constants + scan inputs issued up front so
            # DVE/ACT/SP do them during phase A
            xth_t = stage.tile([128, KTILES * B], f16, tag="xth")
            xtl_t = stage.tile([128, KTILES * B], f16, tag="xtl")
            XQ = 4  # x load chunks
            xq = KTILES * B // XQ
            for q in range(XQ):
                nc.sync.dma_start(xth_t[:, q * xq:(q + 1) * xq], xth_d[:, q * xq:(q + 1) * xq])
                nc.sync.dma_start(xtl_t[:, q * xq:(q + 1) * xq], xtl_d[:, q * xq:(q + 1) * xq])
            b1t = stage.tile([128, NCHUNK], f32, tag="b1t")
            nc.sync.dma_start(b1t[:], b1c_d[:])
            w2s_t = stage.tile([128, NCHUNK, NO], f16, tag="w2s")
            nc.sync.dma_start(w2s_t[:], w2s_d[:])
            b2c_t = stage.tile([NO, 1], f32, tag="b2c")
            nc.sync.dma_start(b2c_t[:], b2c_d[:])
            ident = stage.tile([BLOC, BLOC], f32, tag="ident")
            make_identity(nc, ident[:])
            biasm1 = stage.tile([128, 1], f32, tag="bm1")
            nc.vector.memset(biasm1[:], -1.0)
            zeros_t = stage.tile([128, CB], f32, tag="zeros")
            nc.vector.memset(zeros_t[:], 0.0)
            z5 = stage.tile([NO, BLOC], f32, tag="z5")
            nc.vector.memset(z5[:], 0.0)

            # ---------------- phase A: cur1 partial = xT_slice.T @ W1T_slice
            ps = [[psA.tile([128, 512], f32, tag=f"ps{mb}{nb}", name=f"ps{mb}{nb}")
                   for nb in range(2)] for mb in range(2)]
            xth_v = xth_t[:].rearrange("p (k b) -> p k b", b=B)
            xtl_v = xtl_t[:].rearrange("p (k b) -> p k b", b=B)
            for kb in range(NKB):
                wh_t = wpool.tile([128, KB, HPAD], f16, tag="w1h")
                nc.sync.dma_start(wh_t[:], w1h_d[:, kb * KB:(kb + 1) * KB, :])
                wl_t = wpool.tile([128, KB, HPAD], f16, tag="w1l")
                nc.sync.dma_start(wl_t[:], w1l_d[:, kb * KB:(kb + 1) * KB, :])
                for k2 in range(KB):
                    kt = kb * KB + k2
                    last = kt == KTILES - 1
                    for mb in range(2):
                        xh_s = xth_v[:, kt, mb * 128:(mb + 1) * 128]
                        xl_s = xtl_v[:, kt, mb * 128:(mb + 1) * 128]
                        for nb in range(2):
                            out = ps[mb][nb][:]
                            nc.tensor.matmul(out, xh_s, wl_t[:, k2, nb * 512:(nb + 1) * 512],
                                             start=(kt == 0), stop=False)
                            nc.tensor.matmul(out, xh_s, wh_t[:, k2, nb * 512:(nb + 1) * 512],
                                             start=False, stop=False)
                        for nb in range(2):
                            nc.tensor.matmul(ps[mb][nb][:], xl_s,
                                             wh_t[:, k2, nb * 512:(nb + 1) * 512],
                                             start=False, stop=last)
            partial = dram.tile([B, HPAD], f32)
            for mb in range(2):
                cs = stage.tile([128, HPAD], f32, tag=f"curp{mb}")
                nc.scalar.activation(cs[:, 0:512], ps[mb][0][:],
                                     mybir.ActivationFunctionType.Copy, scale=1.0 / W1SCALE)
                nc.scalar.activation(cs[:, 512:1024], ps[mb][1][:],
                                     mybir.ActivationFunctionType.Copy, scale=1.0 / W1SCALE)
                nc.sync.dma_start(partial[mb * 128:(mb + 1) * 128, :], cs[:])

            # ---------------- ReduceScatter: each core gets its 32-batch slice
            rs_out = dram.tile([BLOC, HPAD], f32)
            if sim:
                # timing stand-in for the collective (single-core TimelineSim)
                nc.sync.dma_start(rs_out[:], partial[0:BLOC, :])
            else:
                nc.gpsimd.collective_compute(
                    "ReduceScatter",
                    mybir.AluOpType.add,
                    replica_groups=[list(range(N_CORES))],
                    ins=[partial.opt()],
                    outs=[rs_out.opt()],
                )

            # ---------------- transpose to scan layout + fold b1
            # curb[p, c*32 + b] = cur1[b, c*128 + p] + b1[c*128 + p]
            rsb = stage.tile([BLOC, HPAD], f32, tag="rsb")
            nc.sync.dma_start(rsb[:], rs_out[:])
            curb = stage.tile([128, CB], f32, tag="curb")
            for c in range(NCHUNK):
                pt = psT.tile([128, BLOC], f32, tag="pst")
                nc.tensor.transpose(pt[:], rsb[:, c * 128:(c + 1) * 128], ident[:])
                nc.scalar.activation(
                    curb[:, c * BLOC:(c + 1) * BLOC], pt[:],
                    mybir.ActivationFunctionType.Identity,
                    bias=b1t[:, c:c + 1], scale=1.0,
                )

            # ---------------- phase B/C: layer-1 scan + layer-2 matmul
            # mem slots: 2 alternating group buffers of 4 step-slots each
            mbuf = [stage.tile([128, G * CB], f32, tag=f"mbuf{g}", name=f"mbuf{g}")
                    for g in range(2)]
            cur2buf = stage.tile([NO, T, BLOC], f32, tag="cur2buf")
            for t in range(1, T + 1):
                gi, sl = (t - 1) // G, (t - 1) % G
                gen = mbuf[gi % 2]
                if t == 1:
                    prev = zeros_t[:]
                elif sl == 0:
                    prev = mbuf[(gi - 1) % 2][:, (G - 1) * CB:]
                else:
                    prev = gen[:, (sl - 1) * CB:sl * CB]
                nc.vector._custom_dve(LIF, out=gen[:, sl * CB:(sl + 1) * CB],
                                      in0=prev, in1=curb[:], s0=BETA)
                if sl == G - 1:
                    # one Sign over the whole group: stream order (sl, c, b)
                    gt = gpool.tile([128, NCHUNK, G, BLOC], f16, tag="gt")
                    nc.scalar.activation(
                        gt[:].rearrange("p c s b -> p s c b"),
                        gen[:].rearrange("p (s c b) -> p s c b", c=NCHUNK, b=BLOC),
                        mybir.ActivationFunctionType.Sign, bias=biasm1[:], scale=1.0,
                    )
                    pc = psC.tile([NO, G * BLOC], f32, tag="psc")
                    for c in range(NCHUNK):
                        nc.tensor.matmul(pc[:], w2s_t[:, c, :],
                                         gt[:, c, :, :].rearrange("p s b -> p (s b)"),
                                         start=(c == 0), stop=(c == NCHUNK - 1))
                    # cur2 = 0.5*(g @ W2T) + (b2 + 0.5*sum W2)  [folds spk=(1+g)/2]
                    nc.scalar.activation(
                        cur2buf[:, gi * G:(gi + 1) * G, :],
                        pc[:].rearrange("o (s b) -> o s b", b=BLOC),
                        mybir.ActivationFunctionType.Identity,
                        bias=b2c_t[:], scale=0.5,
                    )

            # ---------------- phase D: layer-2 scan on [NO, BLOC]
            mem2 = stage.tile([NO, T, BLOC], f32, tag="mem2")
            for t in range(T):
                in0 = z5[:] if t == 0 else mem2[:, t - 1, :]
                nc.vector._custom_dve(
                    LIF,
                    out=mem2[:, t, :],
                    in0=in0,
                    in1=cur2buf[:, t, :],
                    s0=BETA,
                )
            spk2 = stage.tile([NO, T * BLOC], f32, tag="spk2")
            nc.vector.tensor_scalar(spk2[:], mem2[:].rearrange("o t b -> o (t b)"),
                                    1.0, None, mybir.AluOpType.is_gt)
            nc.sync.dma_start(mem2_d[:], mem2[:].rearrange("o t b -> o (t b)"))
            nc.sync.dma_start(spk2_d[:], spk2[:])

    nc.compile()
    _PROGRAMS[sim] = (nc, LIF)
    return _PROGRAMS[sim]


# ---------------------------------------------------------------- host prep
def _prep_inputs(x, W1, b1, W2, b2):
    f32 = np.float32
    x_flat = np.ascontiguousarray(x.reshape(B, -1).astype(f32, copy=False))  # [256, 32000]
    xT = np.zeros((KPAD, B), f32)
    xT[:NIN] = x_flat.T
    xTh = xT.astype(np.float16)
    xTl = (xT - xTh.astype(f32)).astype(np.float16)
    w1T = np.zeros((KPAD, HPAD), f32)
    w1T[:NIN, :NH] = W1.astype(f32, copy=False).T * W1SCALE
    w1Th = w1T.astype(np.float16)
    w1Tl = (w1T - w1Th.astype(f32)).astype(np.float16)
    b1p = np.full(HPAD, -10.0, f32)
    b1p[:NH] = b1
    b1c = np.ascontiguousarray(b1p.reshape(NCHUNK, 128).T)          # [128, 8]
    W2e = np.zeros((HPAD, NO), f32)
    W2e[:NH] = W2.astype(f32, copy=False).T
    w2s = np.ascontiguousarray(
        W2e.reshape(NCHUNK, 128, NO).transpose(1, 0, 2)).astype(np.float16)  # [128,8,5]
    b2c = (b2.astype(f32) + 0.5 * W2.astype(f32).sum(axis=1)).reshape(NO, 1).astype(f32)

    in_maps = []
    for c in range(N_CORES):
        ksl = slice(c * KC, (c + 1) * KC)
        # [4096, X] -> [32, 128, X] -> [128, 32, X] so per-partition DMA rows
        # are contiguous
        xh = np.ascontiguousarray(
            xTh[ksl].reshape(KTILES, 128, B).transpose(1, 0, 2)).reshape(128, KTILES * B)
        xl = np.ascontiguousarray(
            xTl[ksl].reshape(KTILES, 128, B).transpose(1, 0, 2)).reshape(128, KTILES * B)
        wh = np.ascontiguousarray(
            w1Th[ksl].reshape(KTILES, 128, HPAD).transpose(1, 0, 2))
        wl = np.ascontiguousarray(
            w1Tl[ksl].reshape(KTILES, 128, HPAD).transpose(1, 0, 2))
        in_maps.append({
            "xth": xh,
            "xtl": xl,
            "w1h": wh,
            "w1l": wl,
            "b1c": b1c,
            "w2s": w2s,
            "b2c": b2c,
        })
    return in_maps


def _gather(results):
    spk_parts, mem_parts = [], []
    for r in results:
        mem_parts.append(r["mem2rec"].reshape(NO, T, BLOC).transpose(1, 2, 0))
        spk_parts.append(r["spk2rec"].reshape(NO, T, BLOC).transpose(1, 2, 0))
    mem2 = np.concatenate(mem_parts, axis=1).astype(np.float32)  # [200, 256, 5]
    spk2 = np.concatenate(spk_parts, axis=1).astype(np.float32)
    return spk2, mem2


def run_raw(inputs, **kwargs):
    """Build+run; returns BassKernelResults (for profiling from test.py)."""
    from concourse.bass_utils import run_bass_kernel_spmd

    nc, _ = _build_program()
    in_maps = _prep_inputs(**inputs)
    return run_bass_kernel_spmd(nc, in_maps, core_ids=list(range(N_CORES)), **kwargs)


def kernel(x, W1, b1, W2, b2):
    res = run_raw(dict(x=x, W1=W1, b1=b1, W2=W2, b2=b2))
    return _gather(res.results)


if __name__ == "__main__":
    rng = np.random.default_rng(0)
    ins = {
        "x": rng.standard_normal((B, 2, 80, 200)).astype(np.float32),
        "W1": rng.uniform(-1, 1, (NH, NIN)).astype(np.float32) / np.sqrt(NIN),
        "b1": rng.uniform(-1, 1, NH).astype(np.float32) / np.sqrt(NIN),
        "W2": rng.uniform(-1, 1, (NO, NH)).astype(np.float32) / np.sqrt(NH),
        "b2": rng.uniform(-1, 1, NO).astype(np.float32) / np.sqrt(NH),
    }
    spk2, mem2 = kernel(**ins)
    print("shapes:", spk2.shape, mem2.shape, spk2.dtype, mem2.dtype)
    print("spk2 mean:", spk2.mean(), "mem2 std:", mem2.std())
